# revision 34
# baseline (speedup 1.0000x reference)
"""Distributed multi-head attention for Trainium2 (8 NeuronCores).

Problem: nn_MultiHeadAttention (B=2, S=2048, D=1024, H=16, HD=64), f32.

Sharding: tensor parallel over heads — core c owns heads {2c, 2c+1}
(feature slice [128c, 128c+128)) and processes BOTH batches for them.
The output projection is sequence-parallel: per-(head,batch) 8-core
AllToAlls exchange 256-token blocks of the per-head attention outputs,
after which core c holds all 16 heads for tokens [256c, 256c+256) of
EACH batch (512 tokens total) and contracts the full 1024 attention
features against Wo. Splitting the exchange 4 ways lets each collective
launch right after its attention quarter; only the last ~13us A2A plus
half the output projection sit on the tail.

Matmuls run in bf16 (f32 PSUM accumulate). Key Trainium2 facts shaping
the implementation (HW-measured here):
  - PE streams 1 col/cycle at 2.4 GHz for bf16; per-instruction overhead
    (~50ns issue + LDWEIGHTS pump) makes WIDE matmuls win: scores and
    attn*V use 1024-col rhs (two 512-token q blocks per instruction),
    halving instruction+LDWEIGHTS count vs 512-col tiles.
  - X tiles are pre-blocked host-side to [128, ECH, TB] contiguous DRAM
    so every X load is 128 fat 8KB descriptors (not 1024x1KB gathers).
  - Every dma_start costs ~0.6us of sequencer time on the issuing
    engine; steady-state X loads go on Sync, collectives on GpSimd,
    a2a-in on Vector, ao-out on Sync (last one on Scalar, post-exp).
  - ScalarE does ONLY exp (switching activation functions reloads LUTs);
    exp runs on [128, 1024] PSUM tiles to amortize ~250ns ACT overhead.
  - attn^T = V_aug.T @ exp accumulated over k tiles, where V_aug carries
    a ones column -> psum row 64 is the softmax denominator for free.
    The [65,1024] accumulator is bounced to SBUF f32 by VectorE right
    after the last accumulate so the PSUM bank frees in ~1.1us;
    normalization then runs off the SBUF copy.
  - No max subtraction in softmax: scores ~ N(0,1) by construction.
  - PE p-state ramps to 2.4GHz only after ~3us of continuous work, so
    the front of the schedule keeps the PE fed (deep x-tile buffering,
    attention interleaved into projection DMA stalls).
"""

import numpy as np

B = 2
S = 2048          # both n_q and k (per batch)
TS = B * S        # combined token axis (4096)
D = 1024          # embed dim
H = 16            # heads
HD = 64           # head dim
N_CORES = 8
GH = 2            # heads per core
GF = GH * HD      # 128 per-core head features
TB = 512          # per-core output token count (256 per batch)
TBH = 256         # per-batch half-slice
NB = TS // TB     # 8 x blocks
NKT = S // 128    # 16 k tiles per batch
ECH = D // 128    # 8 contraction chunks of the embed dim
QP = 1024         # q columns per attention matmul (2 blocks)

_CACHE = {}
MM_DTYPE = "bf16"  # "bf16" or "f32r"


def _build():
    import concourse.bacc as bacc
    import concourse.tile as tile
    from concourse import mybir

    F32 = mybir.dt.float32
    MDT = mybir.dt.bfloat16 if MM_DTYPE == "bf16" else mybir.dt.float32r
    Act = mybir.ActivationFunctionType

    nc = bacc.Bacc("TRN2", target_bir_lowering=False, debug=False,
                   num_devices=N_CORES)

    # ---- kernel I/O ----
    xq_d = nc.dram_tensor("xq", [NB, 128, ECH, TB], MDT, kind="ExternalInput")
    xk_d = nc.dram_tensor("xk", [NB, 128, ECH, TB], MDT, kind="ExternalInput")
    xv_d = nc.dram_tensor("xv", [NB, 128, ECH, TB], MDT, kind="ExternalInput")
    wqT = nc.dram_tensor("wqT", [128, ECH, GF], MDT, kind="ExternalInput")
    wkT = nc.dram_tensor("wkT", [128, ECH, GF], MDT, kind="ExternalInput")
    wvT = nc.dram_tensor("wvT", [128, ECH, GF], MDT, kind="ExternalInput")
    woT = nc.dram_tensor("woT", [128, ECH, D], MDT, kind="ExternalInput")

    bq_d = nc.dram_tensor("bq", [128, 1], F32, kind="ExternalInput")
    bk_d = nc.dram_tensor("bk", [128, 1], F32, kind="ExternalInput")
    kmask_d = nc.dram_tensor("kmask", [128, GH], F32, kind="ExternalInput")
    kbm_d = nc.dram_tensor("kbm", [128, GH], F32, kind="ExternalInput")
    bv_d = nc.dram_tensor("bv", [128, TB], F32, kind="ExternalInput")
    bo_d = nc.dram_tensor("bo", [128, D], F32, kind="ExternalInput")
    out_d = nc.dram_tensor("out", [TB, D], F32, kind="ExternalOutput")

    groups = [list(range(N_CORES))]

    with tile.TileContext(nc) as tc:
        with (
            tc.tile_pool(name="wpool", bufs=1) as wpool,
            tc.tile_pool(name="state", bufs=1) as state,
            tc.tile_pool(name="xpool", bufs=6) as xpool,
            tc.tile_pool(name="expp", bufs=3) as expp,
            tc.tile_pool(name="pvp", bufs=2) as pvp,
            tc.tile_pool(name="small", bufs=3) as small,
            tc.tile_pool(name="opool", bufs=2) as opool,
            tc.tile_pool(name="ps_proj", bufs=2, space="PSUM") as ps_proj,
            tc.tile_pool(name="ps_sc", bufs=2, space="PSUM") as ps_sc,
            tc.tile_pool(name="ps_at", bufs=1, space="PSUM") as ps_at,
            tc.tile_pool(name="dramp", bufs=1, space="DRAM") as dramp,
        ):
            # ---- startup prefetch: X streams serially on the Sync queue
            # (q,k,v per block -> full per-transfer bandwidth, progressive
            # availability); weights on gpsimd; small biases on scalar ----
            wq_sb = wpool.tile([128, ECH, GF], MDT, name="wq_sb")
            nc.gpsimd.dma_start(wq_sb[:], wqT[:])
            wk_sb = wpool.tile([128, ECH, GF], MDT, name="wk_sb")
            nc.gpsimd.dma_start(wk_sb[:], wkT[:])
            wv_sb = wpool.tile([128, ECH, GF], MDT, name="wv_sb")
            nc.gpsimd.dma_start(wv_sb[:], wvT[:])

            bq_sb = wpool.tile([128, 1], F32, name="bq_sb")
            nc.scalar.dma_start(bq_sb[:], bq_d[:])
            bk_sb = wpool.tile([128, 1], F32, name="bk_sb")
            nc.scalar.dma_start(bk_sb[:], bk_d[:])
            kmask_sb = wpool.tile([128, GH], F32, name="kmask_sb")
            nc.scalar.dma_start(kmask_sb[:], kmask_d[:])
            kbm_sb = wpool.tile([128, GH], F32, name="kbm_sb")
            nc.scalar.dma_start(kbm_sb[:], kbm_d[:])
            bv_sb = wpool.tile([128, TB], F32, name="bv_sb")
            nc.scalar.dma_start(bv_sb[:], bv_d[:])
            # Wo/bo prefetched mid-schedule (see below), after the startup
            # DMA crunch but well before the output projection needs them.
            wo_sb = wpool.tile([128, ECH, D], MDT, name="wo_sb")
            bo_sb = wpool.tile([128, D], F32, name="bo_sb")

            # ---- long-lived state ----
            QT = state.tile([128, TS], MDT, name="QT")
            AT = state.tile([128, TS], MDT, name="AT")
            # per-head zero-padded KT: rows [64h, 64h+64) hold head h's
            # K features, the other 64 rows stay zero -> scores matmuls
            # run K=128 (2x faster than K=64) with unmasked QT as rhs.
            KTp = [state.tile([128, TS], MDT, name=f"KTp{h}")
                   for h in range(GH)]

            # V: [128 tok, tok-chunk, head, 65]; col 64 = ones
            VT = state.tile([128, B * NKT, GH, HD + 1], MDT, name="VT")
            nc.gpsimd.memset(VT[:, :, :, HD:HD + 1], 1.0)

            # per-batch ao tiles: a shared tile would make the batch-0
            # out-projection falsely wait on the LAST ao DMA (batch 1)
            aob = [state.tile([128, ECH, TBH], MDT, name=f"ao{b}")
                   for b in range(B)]

            # ---- emission helpers (PE stream order == emission order) ----
            XD = {"q": xq_d, "k": xk_d, "v": xv_d}

            def xload(b, t, name):
                """Allocate an x tile and start its (fat, contiguous) DMA.
                K tiles ride the scalar queue so the x stream uses two
                hardware-DGE queues (~2x startup bandwidth); a dma_start
                never waits for data, so it can't block the exp stream.
                CAUTION: with the shared ring pool, only issue a load once
                the previous occupant's consumers are already emitted."""
                xt = xpool.tile([128, ECH, TB], MDT, tag="x",
                                name=f"x{name}{b}{t}")
                # batch-0 K rides the scalar queue (2-queue startup
                # bandwidth; its ring slots free early and no exps run
                # yet, so the WAR wait can't block the exp stream)
                eng = nc.scalar if (b == 0 and name == "k") else nc.sync
                eng.dma_start(xt[:], XD[name][b * (S // TB) + t])
                return xt

            def preload(xtiles, b, ts):
                for t in ts:
                    for name in ("q", "k", "v"):
                        xtiles[(t, name)] = xload(b, t, name)

            def emit_proj_gen(b, xtiles):
                """Generator: yields between small PE quanta so projection
                matmuls can be woven into ACT-paced attention streams.
                xtiles holds pre-issued loads for blocks t=0,1; later blocks
                issue their own loads at emission time."""
                for t in range(S // TB):
                    col = b * S + t * TB
                    csl = slice(col, col + TB)
                    # Q, K -> feature-major; K lands in per-head padded rows
                    for name, w_sb, b_sb in (
                        ("q", wq_sb, bq_sb),
                        ("k", wk_sb, bk_sb),
                    ):
                        xt = xtiles.pop((t, name), None)
                        if xt is None:
                            xt = xload(b, t, name)
                        ps = ps_proj.tile([128, TB], F32, tag="pp",
                                          name=f"ps{name}{b}{t}")
                        for e in range(ECH):
                            nc.tensor.matmul(ps[:], w_sb[:, e, :],
                                             xt[:, e, :],
                                             start=(e == 0),
                                             stop=(e == ECH - 1))
                            if e == 3:
                                yield
                        if name == "q":
                            nc.vector.tensor_scalar_add(QT[:, csl], ps[:],
                                                        b_sb[:])
                        else:
                            for h in range(GH):
                                nc.vector.tensor_scalar(
                                    KTp[h][:, csl], ps[:],
                                    kmask_sb[:, h:h + 1], kbm_sb[:, h:h + 1],
                                    op0=mybir.AluOpType.mult,
                                    op1=mybir.AluOpType.add)
                        # yield only AFTER the evac: a block boundary must
                        # leave QT/KTp/VT fully emitted for attention weaves
                        yield
                    # V -> token-major (4 chunks of 128 tokens share 1 psum)
                    xt = xtiles.pop((t, "v"), None)
                    if xt is None:
                        xt = xload(b, t, "v")
                    psv = ps_proj.tile([128, TB], F32, tag="pp",
                                       name=f"psv{b}{t}")
                    for e in range(ECH):
                        for m in range(4):
                            # NOTE: start=True clears has_written for the
                            # WHOLE psum bank, so only the very first matmul
                            # into this bank may set it.
                            nc.tensor.matmul(
                                psv[:, m * GF:(m + 1) * GF],
                                xt[:, e, m * 128:(m + 1) * 128],
                                wv_sb[:, e, :],
                                start=(e == 0 and m == 0),
                                stop=(e == ECH - 1 and m == 3))
                        if e < ECH - 1:
                            yield
                    kt0 = b * NKT + t * 4
                    nc.vector.tensor_add(
                        VT[:, kt0:kt0 + 4, :, 0:HD],
                        psv[:].rearrange("p (m h d) -> p m h d", m=4, h=GH),
                        bv_sb[:].rearrange("p (m h d) -> p m h d", m=4, h=GH))
                    yield

            a2a_in = [[dramp.tile([N_CORES, HD, TBH], MDT,
                                  name=f"a2a_in{h}{b}") for b in range(B)]
                      for h in range(GH)]
            a2a_out = [[dramp.tile([N_CORES, HD, TBH], MDT,
                                   name=f"a2a_out{h}{b}") for b in range(B)]
                       for h in range(GH)]

            def pump(filler, n=1):
                if filler is None:
                    return
                for _ in range(n):
                    try:
                        next(filler)
                    except StopIteration:
                        break

            def emit_attn_gen(h, b):
                """Attention for (head h, batch b): yields once per k-tile
                (32 yields) so the caller can weave in other PE work. Each
                qp epilogue normalizes and immediately streams its half of
                the a2a input to DRAM (overlapped under the next qp)."""
                off = HD * h
                for qp in range(2):
                    qcol = b * S + qp * QP
                    qsl = slice(qcol, qcol + QP)
                    pa = ps_at.tile([HD + 1, QP], F32, tag="at",
                                    name=f"pa{h}{b}{qp}")
                    exps = []
                    for kt in range(NKT):
                        kcol = b * S + kt * 128
                        # one k-chunk stationary, both 512-col q halves
                        # (matmul output must stay within one PSUM bank)
                        pssc = ps_sc.tile([128, QP], F32, tag="sc",
                                          name=f"pssc{h}{b}{qp}{kt}")
                        for i in range(2):
                            nc.tensor.matmul(
                                pssc[:, i * TB:(i + 1) * TB],
                                KTp[h][:, kcol:kcol + 128],
                                QT[:, qcol + i * TB:qcol + (i + 1) * TB],
                                start=True, stop=True)
                        ex = expp.tile([128, QP], MDT, tag="exp",
                                       name=f"ex{h}{b}{qp}{kt}")
                        nc.scalar.activation(ex[:], pssc[:], Act.Exp,
                                             scale=0.125)
                        exps.append(ex)
                        if kt >= 1:
                            for i in range(2):
                                nc.tensor.matmul(
                                    pa[:, i * TB:(i + 1) * TB],
                                    VT[:, b * NKT + kt - 1, h, :],
                                    exps[kt - 1][:, i * TB:(i + 1) * TB],
                                    start=(kt == 1), stop=False)
                        yield
                    for i in range(2):
                        nc.tensor.matmul(
                            pa[:, i * TB:(i + 1) * TB],
                            VT[:, b * NKT + NKT - 1, h, :],
                            exps[NKT - 1][:, i * TB:(i + 1) * TB],
                            start=False, stop=True)
                    # bounce the [65, QP] accumulator to SBUF f32 so the
                    # PSUM banks free fast; normalize off the copy.
                    last = (h == 1 and b == 1 and qp == 1)
                    if last:
                        # tail epilogue is the critical path: no bounce (no
                        # next PSUM user), den copied straight from PSUM
                        src = pa
                    else:
                        src = pvp.tile([HD + 1, QP], F32, tag="pv",
                                       name=f"pv{h}{b}{qp}")
                        nc.vector.tensor_copy(src[:], pa[:])
                    # den must sit at partition 0 for partition_broadcast
                    dn = small.tile([1, QP], F32, tag="dn",
                                    name=f"dn{h}{b}{qp}")
                    nc.vector.tensor_copy(dn[:], src[HD:HD + 1, :])
                    bc = small.tile([HD, QP], F32, tag="bc",
                                    name=f"bc{h}{b}{qp}")
                    nc.gpsimd.partition_broadcast(bc[:], dn[:])
                    # bf16 reciprocal: 2x DVE throughput; den ~1e3 so the
                    # 0.4% rounding is far inside the error budget
                    rc = small.tile([HD, QP], MDT, tag="rc",
                                    name=f"rc{h}{b}{qp}")
                    with nc.allow_low_precision(
                            reason="1/den in bf16: 0.4% rounding on a "
                                   "well-conditioned positive sum"):
                        nc.vector.reciprocal(rc[:], bc[:])
                    nc.vector.tensor_mul(AT[off:off + HD, qsl],
                                         src[0:HD, :], rc[:])
                    # stream this qp's 4 a2a chunks out now. GpSimd software
                    # DGE costs ~10us of ucode per transfer but runs on its
                    # own queue, so mid-kernel halves hide there (sync would
                    # head-of-line block x loads, scalar would block exps).
                    # The last unit sits on the critical path (and a slow
                    # gpsimd gen would also stall the final bcast behind
                    # it): both its halves go on scalar (hardware DGE).
                    eng = nc.scalar if (h == 1 and b == 1) else nc.gpsimd
                    eng.dma_start(
                        a2a_in[h][b][4 * qp:4 * qp + 4].rearrange(
                            "j p n -> p j n"),
                        AT[off:off + HD, qsl].rearrange(
                            "p (j n) -> p j n", j=4))

            def emit_a2a(h, b, defer_ao=False):
                nc.gpsimd.collective_compute(
                    "AllToAll",
                    mybir.AluOpType.bypass,
                    replica_groups=groups,
                    ins=[a2a_in[h][b][:]],
                    outs=[a2a_out[h][b][:]],
                )
                if not defer_ao:
                    emit_ao(h, b)

            def emit_ao(h, b):
                # The ao DMA waits on its collective, so its queue position
                # must have nothing behind it that is needed sooner.
                off = HD * h
                eng = nc.scalar if (h == 1 and b == 1) else nc.sync
                eng.dma_start(
                    aob[b][off:off + HD, :, :],
                    a2a_out[h][b][:].rearrange("j p n -> p j n"))

            def emit_outproj_gen(bh):
                """Output projection for batch bh's 256 tokens (2 m-chunks);
                LDW-reuses the ao stationary across both Wo column halves."""
                for m in (2 * bh, 2 * bh + 1):
                    ot = opool.tile([128, D], F32, tag="ot", name=f"ot{m}")
                    pso = [ps_proj.tile([128, 512], F32, tag="pp",
                                        name=f"pso{m}_{fb}")
                           for fb in range(2)]
                    msl = slice((m % 2) * 128, (m % 2) * 128 + 128)
                    for nq in range(ECH):
                        for fb in range(2):
                            nc.tensor.matmul(
                                pso[fb][:], aob[bh][:, nq, msl],
                                wo_sb[:, nq, fb * 512:(fb + 1) * 512],
                                start=(nq == 0), stop=(nq == ECH - 1))
                        if nq % 2 == 1:
                            yield
                    for fb in range(2):
                        nc.vector.tensor_add(
                            ot[:, fb * 512:(fb + 1) * 512], pso[fb][:],
                            bo_sb[:, fb * 512:(fb + 1) * 512])
                    nc.sync.dma_start(out_d[m * 128:(m + 1) * 128, :], ot[:])

            # ---- schedule.
            # Emission-order invariant: every instruction must be emitted
            # AFTER all writers of the data it reads — Tile orders reads
            # only against already-emitted writers. In particular a k-tile
            # group of attn(h,b) goes in only after its K/V blocks' evacs,
            # and an x load only reuses a ring slot whose previous
            # occupant's consumers are already emitted.
            #
            # Batch-0 start is DMA-bound (12MB of X on one queue), so
            # attn(0,0) k-tile groups are woven in as soon as each block's
            # projections exist: exp starts ~25us in instead of ~60us.
            xb0 = {}
            preload(xb0, 0, (0, 1))
            g0 = emit_proj_gen(0, xb0)
            pump(g0, 24)                 # blocks t=0,1 (12 quanta each)
            a00 = emit_attn_gen(0, 0)
            pump(a00, 8)                 # qp0 kt0-7 (needs K/V t0,t1 only)
            pump(g0, 12)                 # block t=2
            pump(a00, 4)                 # kt8-11
            pump(g0, 64)                 # block t=3
            xb1 = {}
            preload(xb1, 1, (0, 1))      # b1 x stream chains behind b0's
            pump(a00, 64)                # rest of attn(0,0)
            emit_a2a(0, 0, defer_ao=True)
            nc.scalar.dma_start(wo_sb[:], woT[:])
            nc.scalar.dma_start(bo_sb[:], bo_d[:])
            g1 = emit_proj_gen(1, xb1)
            a10 = emit_attn_gen(1, 0)
            for idx in range(64):        # weave b1 proj (48q) into attn(1,0)
                pump(a10, 1)
                pump(g1, 2 if idx % 2 == 0 else 1)
            pump(g1, 64)
            emit_a2a(1, 0, defer_ao=True)
            # ao(0,0)/(1,0) go on sync only now: every x load is already
            # queued, so their wait-on-collective blocks nothing.
            emit_ao(0, 0)
            emit_ao(1, 0)
            for _ in emit_attn_gen(0, 1):
                pass
            emit_a2a(0, 1)
            for _ in emit_attn_gen(1, 1):
                pass
            emit_a2a(1, 1)
            # batch-0 out-proj deps (ao b0, wo) are met long before this
            # point, so it runs on the PE immediately after the last AV,
            # hidden under the final normalize + a2a-in + collective.
            # Emitting it as an attn(1,1) filler instead would head-of-line
            # block the PE whenever a straggler slows collective (1,0).
            for _ in emit_outproj_gen(0):
                pass
            for _ in emit_outproj_gen(1):
                pass

    nc.compile()
    return nc


def _mm_np_dtype():
    if MM_DTYPE == "bf16":
        import ml_dtypes
        return np.dtype(ml_dtypes.bfloat16)
    return np.float32


def _prep_inputs(Q_input, K_input, V_input, Wq, bq, Wk, bk, Wv, bv, Wo, bo):
    """Build the 8 per-core input maps (host-side sharding + transposes)."""
    f32 = np.float32
    mmdt = _mm_np_dtype()
    xblk = {}
    for nm, x in (("xq", Q_input), ("xk", K_input), ("xv", V_input)):
        x = np.asarray(x, f32)
        xf = np.concatenate([x[b].T for b in range(B)], axis=1)  # [D, TS]
        # [NB, 128, ECH, TB]: block bt, partition p, chunk e, token n
        xblk[nm] = np.ascontiguousarray(
            xf.reshape(ECH, 128, NB, TB).transpose(2, 1, 0, 3).astype(mmdt))
    Wq, Wk, Wv, Wo = (np.asarray(w, f32) for w in (Wq, Wk, Wv, Wo))
    bq, bk, bv, bo = (np.asarray(v, f32) for v in (bq, bk, bv, bo))

    def peF(wT):  # [D, F] -> [128, ECH, F] partition-major (fat descriptors)
        return np.ascontiguousarray(
            wT.reshape(ECH, 128, wT.shape[1]).transpose(1, 0, 2).astype(mmdt))

    woT_full = peF(Wo.T)
    bo_bc = np.ascontiguousarray(np.broadcast_to(bo, (128, D)))
    kmask = np.zeros((128, GH), f32)
    for h in range(GH):
        kmask[HD * h:HD * h + HD, h] = 1.0

    in_maps = []
    for c in range(N_CORES):
        hsl = slice(c * GF, (c + 1) * GF)
        in_maps.append({
            **xblk,
            "wqT": peF(Wq[hsl, :].T),
            "wkT": peF(Wk[hsl, :].T),
            "wvT": peF(Wv[hsl, :].T),
            "woT": woT_full,
            "bq": np.ascontiguousarray(bq[hsl].reshape(128, 1)),
            "bk": np.ascontiguousarray(bk[hsl].reshape(128, 1)),
            "kmask": kmask,
            "kbm": np.ascontiguousarray(kmask * bk[hsl].reshape(128, 1)),
            "bv": np.ascontiguousarray(
                np.broadcast_to(np.tile(bv[hsl], 4), (128, TB))),
            "bo": bo_bc,
        })
    return in_maps


def kernel(**inputs):
    from concourse.bass_utils import run_bass_kernel_spmd

    if "nc" not in _CACHE:
        _CACHE["nc"] = _build()
    nc = _CACHE["nc"]

    in_maps = _prep_inputs(**inputs)
    res = run_bass_kernel_spmd(nc, in_maps, core_ids=list(range(N_CORES)))

    out = np.empty((B, S, D), np.float32)
    for c in range(N_CORES):
        r = res.results[c]["out"]
        out[0, TBH * c:TBH * (c + 1), :] = r[0:TBH]
        out[1, TBH * c:TBH * (c + 1), :] = r[TBH:TB]
    return out


# revision 35
# speedup vs baseline: 1.1226x; 1.1226x over previous
"""Distributed multi-head attention for Trainium2 (8 NeuronCores).

Problem: nn_MultiHeadAttention (B=2, S=2048, D=1024, H=16, HD=64), f32.

Sharding: tensor parallel over heads — core c owns heads {2c, 2c+1}
(feature slice [128c, 128c+128)) and processes BOTH batches for them.
The output projection is sequence-parallel: per-(head,batch) 8-core
AllToAlls exchange 256-token blocks of the per-head attention outputs,
after which core c holds all 16 heads for tokens [256c, 256c+256) of
EACH batch (512 tokens total) and contracts the full 1024 attention
features against Wo. Splitting the exchange 4 ways lets each collective
launch right after its attention quarter; only the last ~13us A2A plus
half the output projection sit on the tail.

Matmuls run in bf16 (f32 PSUM accumulate). Key Trainium2 facts shaping
the implementation (HW-measured here):
  - PE streams 1 col/cycle at 2.4 GHz for bf16; per-instruction overhead
    (~50ns issue + LDWEIGHTS pump) makes WIDE matmuls win: scores and
    attn*V use 1024-col rhs (two 512-token q blocks per instruction),
    halving instruction+LDWEIGHTS count vs 512-col tiles.
  - X tiles are pre-blocked host-side to [128, ECH, TB] contiguous DRAM
    so every X load is 128 fat 8KB descriptors (not 1024x1KB gathers).
  - Every dma_start costs ~0.6us of sequencer time on the issuing
    engine; steady-state X loads go on Sync, collectives on GpSimd,
    a2a-in on Vector, ao-out on Sync (last one on Scalar, post-exp).
  - ScalarE does ONLY exp (switching activation functions reloads LUTs);
    exp runs on [128, 1024] PSUM tiles to amortize ~250ns ACT overhead.
  - attn^T = V_aug.T @ exp accumulated over k tiles, where V_aug carries
    a ones column -> psum row 64 is the softmax denominator for free.
    The [65,1024] accumulator is bounced to SBUF f32 by VectorE right
    after the last accumulate so the PSUM bank frees in ~1.1us;
    normalization then runs off the SBUF copy.
  - No max subtraction in softmax: scores ~ N(0,1) by construction.
  - PE p-state ramps to 2.4GHz only after ~3us of continuous work, so
    the front of the schedule keeps the PE fed (deep x-tile buffering,
    attention interleaved into projection DMA stalls).
"""

import numpy as np

B = 2
S = 2048          # both n_q and k (per batch)
TS = B * S        # combined token axis (4096)
D = 1024          # embed dim
H = 16            # heads
HD = 64           # head dim
N_CORES = 8
GH = 2            # heads per core
GF = GH * HD      # 128 per-core head features
TB = 512          # per-core output token count (256 per batch)
TBH = 256         # per-batch half-slice
NB = TS // TB     # 8 x blocks
NKT = S // 128    # 16 k tiles per batch
ECH = D // 128    # 8 contraction chunks of the embed dim
QP = 1024         # q columns per attention matmul (2 blocks)

_CACHE = {}
MM_DTYPE = "bf16"  # "bf16" or "f32r"


def _build():
    import concourse.bacc as bacc
    import concourse.tile as tile
    from concourse import mybir

    F32 = mybir.dt.float32
    MDT = mybir.dt.bfloat16 if MM_DTYPE == "bf16" else mybir.dt.float32r
    Act = mybir.ActivationFunctionType

    nc = bacc.Bacc("TRN2", target_bir_lowering=False, debug=False,
                   num_devices=N_CORES)

    # ---- kernel I/O ----
    xq_d = nc.dram_tensor("xq", [NB, 128, ECH, TB], MDT, kind="ExternalInput")
    xk_d = nc.dram_tensor("xk", [NB, 128, ECH, TB], MDT, kind="ExternalInput")
    xv_d = nc.dram_tensor("xv", [NB, 128, ECH, TB], MDT, kind="ExternalInput")
    wqT = nc.dram_tensor("wqT", [128, ECH, GF], MDT, kind="ExternalInput")
    wkT = nc.dram_tensor("wkT", [128, ECH, GF], MDT, kind="ExternalInput")
    wvT = nc.dram_tensor("wvT", [128, ECH, GF], MDT, kind="ExternalInput")
    woT = nc.dram_tensor("woT", [128, ECH, D], MDT, kind="ExternalInput")

    bq_d = nc.dram_tensor("bq", [128, 1], F32, kind="ExternalInput")
    bk_d = nc.dram_tensor("bk", [128, 1], F32, kind="ExternalInput")
    kmask_d = nc.dram_tensor("kmask", [128, GH], F32, kind="ExternalInput")
    kbm_d = nc.dram_tensor("kbm", [128, GH], F32, kind="ExternalInput")
    bv_d = nc.dram_tensor("bv", [128, TB], F32, kind="ExternalInput")
    bo_d = nc.dram_tensor("bo", [128, D], F32, kind="ExternalInput")
    out_d = nc.dram_tensor("out", [TB, D], F32, kind="ExternalOutput")

    groups = [list(range(N_CORES))]

    with tile.TileContext(nc) as tc:
        with (
            tc.tile_pool(name="wpool", bufs=1) as wpool,
            tc.tile_pool(name="state", bufs=1) as state,
            tc.tile_pool(name="xpool", bufs=6) as xpool,
            tc.tile_pool(name="expp", bufs=3) as expp,
            tc.tile_pool(name="pvp", bufs=2) as pvp,
            tc.tile_pool(name="small", bufs=3) as small,
            tc.tile_pool(name="opool", bufs=2) as opool,
            tc.tile_pool(name="ps_proj", bufs=2, space="PSUM") as ps_proj,
            tc.tile_pool(name="ps_sc", bufs=2, space="PSUM") as ps_sc,
            tc.tile_pool(name="ps_at", bufs=1, space="PSUM") as ps_at,
            tc.tile_pool(name="dramp", bufs=1, space="DRAM") as dramp,
        ):
            # ---- startup prefetch: X streams serially on the Sync queue
            # (q,k,v per block -> full per-transfer bandwidth, progressive
            # availability); weights on gpsimd; small biases on scalar ----
            wq_sb = wpool.tile([128, ECH, GF], MDT, name="wq_sb")
            nc.gpsimd.dma_start(wq_sb[:], wqT[:])
            wk_sb = wpool.tile([128, ECH, GF], MDT, name="wk_sb")
            nc.gpsimd.dma_start(wk_sb[:], wkT[:])
            wv_sb = wpool.tile([128, ECH, GF], MDT, name="wv_sb")
            nc.gpsimd.dma_start(wv_sb[:], wvT[:])

            bq_sb = wpool.tile([128, 1], F32, name="bq_sb")
            nc.scalar.dma_start(bq_sb[:], bq_d[:])
            bk_sb = wpool.tile([128, 1], F32, name="bk_sb")
            nc.scalar.dma_start(bk_sb[:], bk_d[:])
            kmask_sb = wpool.tile([128, GH], F32, name="kmask_sb")
            nc.scalar.dma_start(kmask_sb[:], kmask_d[:])
            kbm_sb = wpool.tile([128, GH], F32, name="kbm_sb")
            nc.scalar.dma_start(kbm_sb[:], kbm_d[:])
            bv_sb = wpool.tile([128, TB], F32, name="bv_sb")
            nc.scalar.dma_start(bv_sb[:], bv_d[:])
            # Wo/bo prefetched mid-schedule (see below), after the startup
            # DMA crunch but well before the output projection needs them.
            wo_sb = wpool.tile([128, ECH, D], MDT, name="wo_sb")
            bo_sb = wpool.tile([128, D], F32, name="bo_sb")

            # ---- long-lived state ----
            QT = state.tile([128, TS], MDT, name="QT")
            AT = state.tile([128, TS], MDT, name="AT")
            # per-head zero-padded KT: rows [64h, 64h+64) hold head h's
            # K features, the other 64 rows stay zero -> scores matmuls
            # run K=128 (2x faster than K=64) with unmasked QT as rhs.
            KTp = [state.tile([128, TS], MDT, name=f"KTp{h}")
                   for h in range(GH)]

            # V: [128 tok, tok-chunk, head, 65]; col 64 = ones
            VT = state.tile([128, B * NKT, GH, HD + 1], MDT, name="VT")
            nc.gpsimd.memset(VT[:, :, :, HD:HD + 1], 1.0)

            # per-batch ao tiles: a shared tile would make the batch-0
            # out-projection falsely wait on the LAST ao DMA (batch 1)
            aob = [state.tile([128, ECH, TBH], MDT, name=f"ao{b}")
                   for b in range(B)]

            # ---- emission helpers (PE stream order == emission order) ----
            XD = {"q": xq_d, "k": xk_d, "v": xv_d}

            def xload(b, t, name):
                """Allocate an x tile and start its (fat, contiguous) DMA.
                K tiles ride the scalar queue so the x stream uses two
                hardware-DGE queues (~2x startup bandwidth); a dma_start
                never waits for data, so it can't block the exp stream.
                CAUTION: with the shared ring pool, only issue a load once
                the previous occupant's consumers are already emitted."""
                xt = xpool.tile([128, ECH, TB], MDT, tag="x",
                                name=f"x{name}{b}{t}")
                # NOTE: measured repeatedly — routing any x loads through
                # the scalar queue slows the ACT/exp pipeline; keep all of
                # them on sync even though it serializes the startup stream.
                nc.sync.dma_start(xt[:], XD[name][b * (S // TB) + t])
                return xt

            def preload(xtiles, b, ts):
                for t in ts:
                    for name in ("q", "k", "v"):
                        xtiles[(t, name)] = xload(b, t, name)

            def emit_proj_gen(b, xtiles):
                """Generator: yields between small PE quanta so projection
                matmuls can be woven into ACT-paced attention streams.
                xtiles holds pre-issued loads for blocks t=0,1; later blocks
                issue their own loads at emission time."""
                for t in range(S // TB):
                    col = b * S + t * TB
                    csl = slice(col, col + TB)
                    # Q, K -> feature-major; K lands in per-head padded rows
                    for name, w_sb, b_sb in (
                        ("q", wq_sb, bq_sb),
                        ("k", wk_sb, bk_sb),
                    ):
                        xt = xtiles.pop((t, name), None)
                        if xt is None:
                            xt = xload(b, t, name)
                        ps = ps_proj.tile([128, TB], F32, tag="pp",
                                          name=f"ps{name}{b}{t}")
                        for e in range(ECH):
                            nc.tensor.matmul(ps[:], w_sb[:, e, :],
                                             xt[:, e, :],
                                             start=(e == 0),
                                             stop=(e == ECH - 1))
                            if e == 3:
                                yield
                        if name == "q":
                            nc.vector.tensor_scalar_add(QT[:, csl], ps[:],
                                                        b_sb[:])
                        else:
                            for h in range(GH):
                                nc.vector.tensor_scalar(
                                    KTp[h][:, csl], ps[:],
                                    kmask_sb[:, h:h + 1], kbm_sb[:, h:h + 1],
                                    op0=mybir.AluOpType.mult,
                                    op1=mybir.AluOpType.add)
                        # yield only AFTER the evac: a block boundary must
                        # leave QT/KTp/VT fully emitted for attention weaves
                        yield
                    # V -> token-major (4 chunks of 128 tokens share 1 psum)
                    xt = xtiles.pop((t, "v"), None)
                    if xt is None:
                        xt = xload(b, t, "v")
                    psv = ps_proj.tile([128, TB], F32, tag="pp",
                                       name=f"psv{b}{t}")
                    for e in range(ECH):
                        for m in range(4):
                            # NOTE: start=True clears has_written for the
                            # WHOLE psum bank, so only the very first matmul
                            # into this bank may set it.
                            nc.tensor.matmul(
                                psv[:, m * GF:(m + 1) * GF],
                                xt[:, e, m * 128:(m + 1) * 128],
                                wv_sb[:, e, :],
                                start=(e == 0 and m == 0),
                                stop=(e == ECH - 1 and m == 3))
                        if e < ECH - 1:
                            yield
                    kt0 = b * NKT + t * 4
                    nc.vector.tensor_add(
                        VT[:, kt0:kt0 + 4, :, 0:HD],
                        psv[:].rearrange("p (m h d) -> p m h d", m=4, h=GH),
                        bv_sb[:].rearrange("p (m h d) -> p m h d", m=4, h=GH))
                    yield

            a2a_in = [[dramp.tile([N_CORES, HD, TBH], MDT,
                                  name=f"a2a_in{h}{b}") for b in range(B)]
                      for h in range(GH)]
            a2a_out = [[dramp.tile([N_CORES, HD, TBH], MDT,
                                   name=f"a2a_out{h}{b}") for b in range(B)]
                       for h in range(GH)]

            def pump(filler, n=1):
                if filler is None:
                    return
                for _ in range(n):
                    try:
                        next(filler)
                    except StopIteration:
                        break

            def emit_attn_gen(h, b):
                """Attention for (head h, batch b): yields once per k-tile
                (32 yields) so the caller can weave in other PE work. Each
                qp epilogue normalizes and immediately streams its half of
                the a2a input to DRAM (overlapped under the next qp)."""
                off = HD * h
                for qp in range(2):
                    qcol = b * S + qp * QP
                    qsl = slice(qcol, qcol + QP)
                    pa = ps_at.tile([HD + 1, QP], F32, tag="at",
                                    name=f"pa{h}{b}{qp}")
                    exps = []
                    for kt in range(NKT):
                        kcol = b * S + kt * 128
                        # one k-chunk stationary, both 512-col q halves
                        # (matmul output must stay within one PSUM bank)
                        pssc = ps_sc.tile([128, QP], F32, tag="sc",
                                          name=f"pssc{h}{b}{qp}{kt}")
                        for i in range(2):
                            nc.tensor.matmul(
                                pssc[:, i * TB:(i + 1) * TB],
                                KTp[h][:, kcol:kcol + 128],
                                QT[:, qcol + i * TB:qcol + (i + 1) * TB],
                                start=True, stop=True)
                        ex = expp.tile([128, QP], MDT, tag="exp",
                                       name=f"ex{h}{b}{qp}{kt}")
                        nc.scalar.activation(ex[:], pssc[:], Act.Exp,
                                             scale=0.125)
                        exps.append(ex)
                        if kt >= 1:
                            for i in range(2):
                                nc.tensor.matmul(
                                    pa[:, i * TB:(i + 1) * TB],
                                    VT[:, b * NKT + kt - 1, h, :],
                                    exps[kt - 1][:, i * TB:(i + 1) * TB],
                                    start=(kt == 1), stop=False)
                        yield
                    for i in range(2):
                        nc.tensor.matmul(
                            pa[:, i * TB:(i + 1) * TB],
                            VT[:, b * NKT + NKT - 1, h, :],
                            exps[NKT - 1][:, i * TB:(i + 1) * TB],
                            start=False, stop=True)
                    # bounce the [65, QP] accumulator to SBUF f32 so the
                    # PSUM banks free fast; normalize off the copy.
                    last = (h == 1 and b == 1 and qp == 1)
                    if last:
                        # tail epilogue is the critical path: no bounce (no
                        # next PSUM user), den copied straight from PSUM
                        src = pa
                    else:
                        src = pvp.tile([HD + 1, QP], F32, tag="pv",
                                       name=f"pv{h}{b}{qp}")
                        nc.vector.tensor_copy(src[:], pa[:])
                    # den must sit at partition 0 for partition_broadcast
                    dn = small.tile([1, QP], F32, tag="dn",
                                    name=f"dn{h}{b}{qp}")
                    nc.vector.tensor_copy(dn[:], src[HD:HD + 1, :])
                    bc = small.tile([HD, QP], F32, tag="bc",
                                    name=f"bc{h}{b}{qp}")
                    nc.gpsimd.partition_broadcast(bc[:], dn[:])
                    # bf16 reciprocal: 2x DVE throughput; den ~1e3 so the
                    # 0.4% rounding is far inside the error budget
                    rc = small.tile([HD, QP], MDT, tag="rc",
                                    name=f"rc{h}{b}{qp}")
                    with nc.allow_low_precision(
                            reason="1/den in bf16: 0.4% rounding on a "
                                   "well-conditioned positive sum"):
                        nc.vector.reciprocal(rc[:], bc[:])
                    nc.vector.tensor_mul(AT[off:off + HD, qsl],
                                         src[0:HD, :], rc[:])
                    # stream this qp's 4 a2a chunks out now. GpSimd software
                    # DGE costs ~10us of ucode per transfer but runs on its
                    # own queue, so mid-kernel halves hide there (sync would
                    # head-of-line block x loads, scalar would block exps).
                    # The last unit sits on the critical path (and a slow
                    # gpsimd gen would also stall the final bcast behind
                    # it): both its halves go on scalar (hardware DGE).
                    eng = nc.scalar if (h == 1 and b == 1) else nc.gpsimd
                    eng.dma_start(
                        a2a_in[h][b][4 * qp:4 * qp + 4].rearrange(
                            "j p n -> p j n"),
                        AT[off:off + HD, qsl].rearrange(
                            "p (j n) -> p j n", j=4))

            def emit_a2a(h, b, defer_ao=False):
                nc.gpsimd.collective_compute(
                    "AllToAll",
                    mybir.AluOpType.bypass,
                    replica_groups=groups,
                    ins=[a2a_in[h][b][:]],
                    outs=[a2a_out[h][b][:]],
                )
                if not defer_ao:
                    emit_ao(h, b)

            def emit_ao(h, b):
                # The ao DMA waits on its collective, so its queue position
                # must have nothing behind it that is needed sooner.
                off = HD * h
                eng = nc.scalar if (h == 1 and b == 1) else nc.sync
                eng.dma_start(
                    aob[b][off:off + HD, :, :],
                    a2a_out[h][b][:].rearrange("j p n -> p j n"))

            def emit_outproj_gen(bh):
                """Output projection for batch bh's 256 tokens (2 m-chunks);
                LDW-reuses the ao stationary across both Wo column halves."""
                for m in (2 * bh, 2 * bh + 1):
                    ot = opool.tile([128, D], F32, tag="ot", name=f"ot{m}")
                    pso = [ps_proj.tile([128, 512], F32, tag="pp",
                                        name=f"pso{m}_{fb}")
                           for fb in range(2)]
                    msl = slice((m % 2) * 128, (m % 2) * 128 + 128)
                    for nq in range(ECH):
                        for fb in range(2):
                            nc.tensor.matmul(
                                pso[fb][:], aob[bh][:, nq, msl],
                                wo_sb[:, nq, fb * 512:(fb + 1) * 512],
                                start=(nq == 0), stop=(nq == ECH - 1))
                        if nq % 2 == 1:
                            yield
                    for fb in range(2):
                        nc.vector.tensor_add(
                            ot[:, fb * 512:(fb + 1) * 512], pso[fb][:],
                            bo_sb[:, fb * 512:(fb + 1) * 512])
                    nc.sync.dma_start(out_d[m * 128:(m + 1) * 128, :], ot[:])

            # ---- schedule.
            # Emission-order invariant: every instruction must be emitted
            # AFTER all writers of the data it reads — Tile orders reads
            # only against already-emitted writers. In particular a k-tile
            # group of attn(h,b) goes in only after its K/V blocks' evacs,
            # and an x load only reuses a ring slot whose previous
            # occupant's consumers are already emitted.
            #
            # Batch-0 start is DMA-bound (12MB of X on one queue), so
            # attn(0,0) k-tile groups are woven in as soon as each block's
            # projections exist: exp starts ~25us in instead of ~60us.
            xb0 = {}
            preload(xb0, 0, (0, 1))
            g0 = emit_proj_gen(0, xb0)
            pump(g0, 24)                 # blocks t=0,1 (12 quanta each)
            a00 = emit_attn_gen(0, 0)
            pump(a00, 8)                 # qp0 kt0-7 (needs K/V t0,t1 only)
            pump(g0, 12)                 # block t=2
            pump(a00, 4)                 # kt8-11
            pump(g0, 64)                 # block t=3
            xb1 = {}
            preload(xb1, 1, (0, 1))      # b1 x stream chains behind b0's
            pump(a00, 64)                # rest of attn(0,0)
            emit_a2a(0, 0, defer_ao=True)
            nc.scalar.dma_start(wo_sb[:], woT[:])
            nc.scalar.dma_start(bo_sb[:], bo_d[:])
            g1 = emit_proj_gen(1, xb1)
            a10 = emit_attn_gen(1, 0)
            for idx in range(64):        # weave b1 proj (48q) into attn(1,0)
                pump(a10, 1)
                pump(g1, 2 if idx % 2 == 0 else 1)
            pump(g1, 64)
            emit_a2a(1, 0, defer_ao=True)
            # ao(0,0)/(1,0) go on sync only now: every x load is already
            # queued, so their wait-on-collective blocks nothing.
            emit_ao(0, 0)
            emit_ao(1, 0)
            for _ in emit_attn_gen(0, 1):
                pass
            emit_a2a(0, 1)
            for _ in emit_attn_gen(1, 1):
                pass
            emit_a2a(1, 1)
            # batch-0 out-proj deps (ao b0, wo) are met long before this
            # point, so it runs on the PE immediately after the last AV,
            # hidden under the final normalize + a2a-in + collective.
            # Emitting it as an attn(1,1) filler instead would head-of-line
            # block the PE whenever a straggler slows collective (1,0).
            for _ in emit_outproj_gen(0):
                pass
            for _ in emit_outproj_gen(1):
                pass

    nc.compile()
    return nc


def _mm_np_dtype():
    if MM_DTYPE == "bf16":
        import ml_dtypes
        return np.dtype(ml_dtypes.bfloat16)
    return np.float32


def _prep_inputs(Q_input, K_input, V_input, Wq, bq, Wk, bk, Wv, bv, Wo, bo):
    """Build the 8 per-core input maps (host-side sharding + transposes)."""
    f32 = np.float32
    mmdt = _mm_np_dtype()
    xblk = {}
    for nm, x in (("xq", Q_input), ("xk", K_input), ("xv", V_input)):
        x = np.asarray(x, f32)
        xf = np.concatenate([x[b].T for b in range(B)], axis=1)  # [D, TS]
        # [NB, 128, ECH, TB]: block bt, partition p, chunk e, token n
        xblk[nm] = np.ascontiguousarray(
            xf.reshape(ECH, 128, NB, TB).transpose(2, 1, 0, 3).astype(mmdt))
    Wq, Wk, Wv, Wo = (np.asarray(w, f32) for w in (Wq, Wk, Wv, Wo))
    bq, bk, bv, bo = (np.asarray(v, f32) for v in (bq, bk, bv, bo))

    def peF(wT):  # [D, F] -> [128, ECH, F] partition-major (fat descriptors)
        return np.ascontiguousarray(
            wT.reshape(ECH, 128, wT.shape[1]).transpose(1, 0, 2).astype(mmdt))

    woT_full = peF(Wo.T)
    bo_bc = np.ascontiguousarray(np.broadcast_to(bo, (128, D)))
    kmask = np.zeros((128, GH), f32)
    for h in range(GH):
        kmask[HD * h:HD * h + HD, h] = 1.0

    in_maps = []
    for c in range(N_CORES):
        hsl = slice(c * GF, (c + 1) * GF)
        in_maps.append({
            **xblk,
            "wqT": peF(Wq[hsl, :].T),
            "wkT": peF(Wk[hsl, :].T),
            "wvT": peF(Wv[hsl, :].T),
            "woT": woT_full,
            "bq": np.ascontiguousarray(bq[hsl].reshape(128, 1)),
            "bk": np.ascontiguousarray(bk[hsl].reshape(128, 1)),
            "kmask": kmask,
            "kbm": np.ascontiguousarray(kmask * bk[hsl].reshape(128, 1)),
            "bv": np.ascontiguousarray(
                np.broadcast_to(np.tile(bv[hsl], 4), (128, TB))),
            "bo": bo_bc,
        })
    return in_maps


def kernel(**inputs):
    from concourse.bass_utils import run_bass_kernel_spmd

    if "nc" not in _CACHE:
        _CACHE["nc"] = _build()
    nc = _CACHE["nc"]

    in_maps = _prep_inputs(**inputs)
    res = run_bass_kernel_spmd(nc, in_maps, core_ids=list(range(N_CORES)))

    out = np.empty((B, S, D), np.float32)
    for c in range(N_CORES):
        r = res.results[c]["out"]
        out[0, TBH * c:TBH * (c + 1), :] = r[0:TBH]
        out[1, TBH * c:TBH * (c + 1), :] = r[TBH:TB]
    return out
